# revision 31
# baseline (speedup 1.0000x reference)
"""Causal self-attention with RoPE on 8 Trainium2 NeuronCores.

Sharding: 2-way data parallel over batch x 4-way tensor parallel over heads
(4 heads per core).  Each core computes, for its (batch, head-group):
    QKV projection (bf16 matmuls, fp32 accumulate) with RoPE applied to Q/K,
    causal attention with softmax (no max-subtraction; scores are O(7)),
    attention-output projection against its slice of w_out rows.
Each core returns a partial (T, D) fp32 output; the host sums the 4
head-group partials per batch element (the "all-reduce after out_proj").

Layout choices:
  - Q^T / K^T (head_dim on partitions, tokens free) come straight out of the
    projection matmuls; scores are computed transposed S^T = K^T-tiles^T @ Q^T
    so the probabilities are already in the (k, q) layout that the PV matmul
    needs as its stationary operand.
  - V is computed in natural (token, head_dim) layout with a column of ones
    appended per head, so the PV matmul's free dim 129 yields both the
    unnormalized attention output and the softmax normalizer Z per q row.
  - RoPE rotate-half is a fixed 128x128 permutation done on the tensor engine.
"""

import os

import numpy as np
import ml_dtypes

B, T, D = 2, 2048, 2048
H, HD = 16, 128
NH = 4               # heads per core
P = 128
KD = D // P          # 16 contraction tiles over the model dim
TCH = 512            # token chunk (matmul moving free dim)
NTCH = T // TCH      # 4
NTT = T // P         # 16 token tiles
ROPE_BASE = 10000.0
SCALE = float(HD) ** -0.5
BF16 = ml_dtypes.bfloat16

_CACHE = {}
LAST_RESULT = None


def _build_program():
    import concourse.bacc as bacc
    import concourse.mybir as mybir
    import concourse.tile as tile

    f32 = mybir.dt.float32
    bf16 = mybir.dt.bfloat16
    Exp = mybir.ActivationFunctionType.Exp
    Copy = mybir.ActivationFunctionType.Copy

    nc = bacc.Bacc("TRN2")
    xT = nc.dram_tensor("xT", (D, T), bf16, kind="ExternalInput").ap()
    wqk = nc.dram_tensor("wqk", (D, 2 * NH * HD), bf16, kind="ExternalInput").ap()
    wv = nc.dram_tensor("wv", (D, NH * HD), bf16, kind="ExternalInput").ap()
    wout = nc.dram_tensor("wout", (NH * HD, D), bf16, kind="ExternalInput").ap()
    cosT = nc.dram_tensor("cosT", (P, T), bf16, kind="ExternalInput").ap()
    sinT = nc.dram_tensor("sinT", (P, T), bf16, kind="ExternalInput").ap()
    tri = nc.dram_tensor("tri", (P, P), bf16, kind="ExternalInput").ap()
    iden = nc.dram_tensor("iden", (P, P), bf16, kind="ExternalInput").ap()
    swap = nc.dram_tensor("swap", (P, P), bf16, kind="ExternalInput").ap()
    out = nc.dram_tensor("out", (T, D), f32, kind="ExternalOutput").ap()

    VW = HD + 1          # per-head V width incl. the ones column

    with tile.TileContext(nc) as tc:
        with tc.tile_pool(name="const", bufs=1) as const:
            # Constant preloads ride the gpsimd SWDGE queue so they overlap
            # the x-tile streaming on the sync HWDGE queue; split the big
            # ones so the first k-slices land within a couple of us and the
            # first matmuls can start almost immediately.
            # Preloads spread over three per-engine SWDGE queues (gpsimd /
            # scalar / vector) so they stream in parallel with the x tiles on
            # the sync HWDGE queue.  wqk is split per k-slice so the first
            # matmul can start ~1us in and slices arrive at consumption rate.
            wqk_r = wqk.rearrange("(ko p) c -> p ko c", p=P)
            wqk_sb = const.tile([P, KD, 2 * NH * HD], bf16)
            for k in range(KD):
                nc.gpsimd.dma_start(wqk_sb[:, k, :], wqk_r[:, k, :])
            cos_sb = const.tile([P, T], bf16)
            nc.scalar.dma_start(cos_sb, cosT)
            sin_sb = const.tile([P, T], bf16)
            nc.scalar.dma_start(sin_sb, sinT)
            tri_sb = const.tile([P, P], bf16)
            nc.scalar.dma_start(tri_sb, tri)
            id_sb = const.tile([P, P], bf16)
            nc.scalar.dma_start(id_sb, iden)
            # wv / wout are loaded lazily (below) so the startup HBM burst is
            # only what the first token-chunk's matmuls actually need.
            wv_r = wv.rearrange("(ko p) c -> p ko c", p=P)
            wv_sb = const.tile([P, KD, NH * HD], bf16)
            wout_sb = const.tile([P, NH, D], bf16)

            qt_sb = const.tile([P, NH, T], bf16)    # Q^T per head (rope'd)
            kt_sb = const.tile([P, NH, T], bf16)    # K^T per head (rope'd)
            vp_sb = const.tile([P, NTT, NH * VW], bf16)  # V' per token tile
            at_sb = const.tile([P, NH, T], bf16)    # attn output transposed

            for h in range(NH):
                nc.vector.memset(vp_sb[:, :, h * VW + HD : h * VW + VW], 1.0)

            # ---------------- Stage A: projections + RoPE ----------------
            with tc.tile_pool(name="xp", bufs=KD + 6) as xpool, \
                 tc.tile_pool(name="qkps", bufs=6, space="PSUM") as qk_ps, \
                 tc.tile_pool(name="vps", bufs=2, space="PSUM") as v_ps, \
                 tc.tile_pool(name="ropet", bufs=2) as rp:
                H2 = HD // 2

                def rope_epilogue(ps, m, tsl):
                    # rotate-half via two partition-shifted DVE multiplies
                    # (sign already folded into sin_sb)
                    cp = rp.tile([P, TCH], f32, tag="cp")
                    nc.vector.tensor_mul(cp, ps, cos_sb[:, tsl])
                    sp = rp.tile([P, TCH], f32, tag="sp")
                    nc.vector.tensor_mul(sp[:H2], ps[H2:], sin_sb[:H2, tsl])
                    nc.vector.tensor_mul(sp[H2:], ps[:H2], sin_sb[H2:, tsl])
                    dst = qt_sb[:, m, tsl] if m < NH else kt_sb[:, m - NH, tsl]
                    nc.vector.tensor_add(dst, sp, cp)

                for tci in range(NTCH):
                    tsl = slice(tci * TCH, (tci + 1) * TCH)
                    xts = []
                    for k in range(KD):
                        xt = xpool.tile([P, TCH], bf16, tag="x")
                        # chunks 0-1: keep the sync HWDGE queue to itself
                        # (gpsimd is still streaming wqk); later chunks split.
                        eng = nc.sync if (tci <= 1 or k % 2 == 0) else nc.gpsimd
                        eng.dma_start(xt, xT[k * P : (k + 1) * P, tsl])
                        xts.append(xt)
                    if tci == 0:
                        # k-outer on the first chunk: consume wqk/x k-slices
                        # in DMA arrival order instead of stalling on the
                        # full 4MB of wqk inside the first m's k-loop.
                        for half in range(2):
                            pss = [
                                qk_ps.tile([P, TCH], f32, tag="qk", name=f"qk{i}")
                                for i in range(NH)
                            ]
                            for k in range(KD):
                                for mi in range(NH):
                                    m = half * NH + mi
                                    nc.tensor.matmul(
                                        pss[mi],
                                        wqk_sb[:, k, m * HD : (m + 1) * HD],
                                        xts[k],
                                        start=(k == 0),
                                        stop=(k == KD - 1),
                                    )
                            for mi in range(NH):
                                rope_epilogue(pss[mi], half * NH + mi, tsl)
                    else:
                        for m in range(2 * NH):
                            ps = qk_ps.tile([P, TCH], f32, tag="qk")
                            for k in range(KD):
                                nc.tensor.matmul(
                                    ps,
                                    wqk_sb[:, k, m * HD : (m + 1) * HD],
                                    xts[k],
                                    start=(k == 0),
                                    stop=(k == KD - 1),
                                )
                            rope_epilogue(ps, m, tsl)
                    if tci == 0:
                        for k4 in range(4):
                            nc.scalar.dma_start(
                                wv_sb[:, k4 * 4 : (k4 + 1) * 4, :],
                                wv_r[:, k4 * 4 : (k4 + 1) * 4, :],
                            )
                    for tt in range(TCH // P):
                        ttg = tci * (TCH // P) + tt
                        psv = v_ps.tile([P, NH * HD], f32, tag="v")
                        for k in range(KD):
                            nc.tensor.matmul(
                                psv,
                                xts[k][:, tt * P : (tt + 1) * P],
                                wv_sb[:, k, :],
                                start=(k == 0),
                                stop=(k == KD - 1),
                            )
                        for h in range(NH):
                            nc.vector.tensor_copy(
                                vp_sb[:, ttg, h * VW : h * VW + HD],
                                psv[:, h * HD : (h + 1) * HD],
                            )

            # -------- Stage B+C: causal attention + output projection --------
            # qc outer, heads inner: once all heads finish a q-chunk, its four
            # token tiles of the output projection are emitted immediately so
            # they overlap the next q-chunk's attention work on the PE.
            # PSUM budget: st 2 + pv/o shared 5 + tp 1 = 8 banks.
            with tc.tile_pool(name="stps", bufs=3, space="PSUM") as st_ps, \
                 tc.tile_pool(name="pvps", bufs=4, space="PSUM") as pv_ps, \
                 tc.tile_pool(name="tpps", bufs=1, space="PSUM") as tp_ps, \
                 tc.tile_pool(name="ptp", bufs=8) as ptp, \
                 tc.tile_pool(name="bt", bufs=8) as bt, \
                 tc.tile_pool(name="ost", bufs=4) as ost:
                for h in range(NH):
                    nc.gpsimd.dma_start(wout_sb[:, h, :], wout[h * P : (h + 1) * P, :])
                for qc in range(NTCH):
                    for h in range(NH):
                        pvp = [
                            pv_ps.tile([P, TCH], f32, tag="pv", name=f"pv{i}")
                            for i in range(TCH // P)
                        ]
                        for ki in range(4 * qc + 4):
                            # diagonal blocks: only q >= ki is causally valid,
                            # so trim the matmul/exp to the valid q suffix
                            j = max(ki - 4 * qc, 0) * P
                            st = st_ps.tile([P, TCH], f32, tag="st")
                            nc.tensor.matmul(
                                st[:, j:],
                                kt_sb[:, h, ki * P : (ki + 1) * P],
                                qt_sb[:, h, qc * TCH + j : (qc + 1) * TCH],
                                start=True,
                                stop=True,
                            )
                            pt = ptp.tile([P, TCH], bf16, tag="pt")
                            nc.scalar.activation(pt[:, j:], st[:, j:], Exp, scale=SCALE)
                            for ql in range(TCH // P):
                                qi = 4 * qc + ql
                                if qi < ki:
                                    continue
                                src = pt[:, ql * P : (ql + 1) * P]
                                if qi == ki:
                                    nc.vector.tensor_mul(src, src, tri_sb)
                                nc.tensor.matmul(
                                    pvp[ql][:, : HD + 1],
                                    src,
                                    vp_sb[:, ki, h * VW : (h + 1) * VW],
                                    start=(ki == 0),
                                    stop=(ki == qi),
                                )
                        for ql in range(TCH // P):
                            qi = 4 * qc + ql
                            invz = bt.tile([P, 1], f32, tag="invz")
                            nc.vector.reciprocal(invz, pvp[ql][:, HD : HD + 1])
                            anorm = bt.tile([P, P], bf16, tag="anorm")
                            nc.vector.tensor_scalar_mul(anorm, pvp[ql][:, :HD], invz)
                            tp = tp_ps.tile([P, P], bf16, tag="tp")
                            nc.tensor.transpose(tp, anorm, id_sb)
                            nc.vector.tensor_copy(at_sb[:, h, qi * P : (qi + 1) * P], tp)
                    for tl in range(TCH // P):
                        tti = 4 * qc + tl
                        for oc in range(D // TCH):
                            po = pv_ps.tile([P, TCH], f32, tag="pv", name="po")
                            for h in range(NH):
                                nc.tensor.matmul(
                                    po,
                                    at_sb[:, h, tti * P : (tti + 1) * P],
                                    wout_sb[:, h, oc * TCH : (oc + 1) * TCH],
                                    start=(h == 0),
                                    stop=(h == NH - 1),
                                )
                            ob = ost.tile([P, TCH], f32, tag="ob")
                            # keep ACT exp-only in this phase: copies on DVE,
                            # DMA triggers on sync/gpsimd (both have slack)
                            nc.vector.tensor_copy(ob, po)
                            eng = nc.sync if oc % 2 == 0 else nc.gpsimd
                            eng.dma_start(
                                out[tti * P : (tti + 1) * P, oc * TCH : (oc + 1) * TCH],
                                ob,
                            )

    nc.compile()
    return nc


def _rope_tables():
    inv_freq = 1.0 / (
        np.float32(ROPE_BASE)
        ** (np.arange(0, HD, 2, dtype=np.float32) / np.float32(HD))
    )
    t = np.arange(T, dtype=np.float32)
    freqs = np.einsum("i,j->ij", t, inv_freq).astype(np.float32)   # (T, HD/2)
    emb = np.concatenate([freqs, freqs], axis=-1)                  # (T, HD)
    cos = np.cos(emb).astype(np.float32)
    sin = np.sin(emb).astype(np.float32)
    cosT = np.ascontiguousarray(cos.T)                             # (HD, T)
    sinT = np.ascontiguousarray(sin.T)
    sinT[: HD // 2] *= -1.0                                        # sign-fold rotate_half
    return cosT, sinT


def _make_in_maps(x, w_qkv, w_out):
    cosT, sinT = _rope_tables()
    kk, qq = np.meshgrid(np.arange(P), np.arange(P), indexing="ij")
    tri = (qq >= kk).astype(BF16)
    iden = np.eye(P, dtype=BF16)
    swap = np.zeros((P, P), dtype=BF16)
    swap[np.arange(P), (np.arange(P) + HD // 2) % P] = 1

    in_maps = []
    for c in range(8):
        b, hg = c // 4, c % 4
        cs = slice(hg * NH * HD, (hg + 1) * NH * HD)
        wq = w_qkv[:, 0 * D : 1 * D][:, cs]
        wk = w_qkv[:, 1 * D : 2 * D][:, cs]
        wv = w_qkv[:, 2 * D : 3 * D][:, cs]
        in_maps.append(
            {
                "xT": np.ascontiguousarray(x[b].T).astype(BF16),
                "wqk": np.ascontiguousarray(
                    np.concatenate([wq, wk], axis=1)
                ).astype(BF16),
                "wv": np.ascontiguousarray(wv).astype(BF16),
                "wout": np.ascontiguousarray(w_out[cs, :]).astype(BF16),
                "cosT": cosT.astype(BF16),
                "sinT": sinT.astype(BF16),
                "tri": tri,
                "iden": iden,
                "swap": swap,
            }
        )
    return in_maps


def _ensure_ntff_hook():
    """bass_utils needs antenv.axon_hooks when tracing under axon; some
    images ship an antenv without it.  Recreate the tiny hook registry and
    register the ctypes-based NTFF capture the boot shim would have."""
    try:
        import antenv.axon_hooks  # noqa: F401
        return
    except ImportError:
        pass
    import sys
    import types

    import antenv

    mod = types.ModuleType("antenv.axon_hooks")
    mod._hook = None

    def set_axon_ntff_profile_hook(h):
        mod._hook = h

    def get_axon_ntff_profile_hook():
        return mod._hook

    mod.set_axon_ntff_profile_hook = set_axon_ntff_profile_hook
    mod.get_axon_ntff_profile_hook = get_axon_ntff_profile_hook
    sys.modules["antenv.axon_hooks"] = mod
    antenv.axon_hooks = mod
    try:
        from trn_agent_boot.trn_boot import _ntff_profile_via_ctypes

        mod._hook = _ntff_profile_via_ctypes("/opt/axon/libaxon_pjrt.so")
    except Exception:
        pass


def kernel(x, w_qkv, w_out):
    global LAST_RESULT
    from concourse import bass_utils

    x = np.asarray(x, dtype=np.float32)
    w_qkv = np.asarray(w_qkv, dtype=np.float32)
    w_out = np.asarray(w_out, dtype=np.float32)

    if "nc" not in _CACHE:
        _CACHE["nc"] = _build_program()
    nc = _CACHE["nc"]

    in_maps = _make_in_maps(x, w_qkv, w_out)
    trace = os.environ.get("KERNEL_TRACE", "0") == "1"
    if trace:
        _ensure_ntff_hook()
    res = bass_utils.run_bass_kernel_spmd(
        nc, in_maps, core_ids=list(range(8)), trace=trace
    )
    LAST_RESULT = res

    out = np.zeros((B, T, D), dtype=np.float32)
    for c in range(8):
        out[c // 4] += res.results[c]["out"]
    return out


# revision 33
# speedup vs baseline: 1.0224x; 1.0224x over previous
"""Causal self-attention with RoPE on 8 Trainium2 NeuronCores.

Sharding: 2-way data parallel over batch x 4-way tensor parallel over heads
(4 heads per core).  Each core computes, for its (batch, head-group):
    QKV projection (bf16 matmuls, fp32 accumulate) with RoPE applied to Q/K,
    causal attention with softmax (no max-subtraction; scores are O(7)),
    attention-output projection against its slice of w_out rows.
Each core returns a partial (T, D) fp32 output; the host sums the 4
head-group partials per batch element (the "all-reduce after out_proj").

Layout choices:
  - Q^T / K^T (head_dim on partitions, tokens free) come straight out of the
    projection matmuls; scores are computed transposed S^T = K^T-tiles^T @ Q^T
    so the probabilities are already in the (k, q) layout that the PV matmul
    needs as its stationary operand.
  - V is computed in natural (token, head_dim) layout with a column of ones
    appended per head, so the PV matmul's free dim 129 yields both the
    unnormalized attention output and the softmax normalizer Z per q row.
  - RoPE rotate-half is a fixed 128x128 permutation done on the tensor engine.
"""

import os

import numpy as np
import ml_dtypes

B, T, D = 2, 2048, 2048
H, HD = 16, 128
NH = 4               # heads per core
P = 128
KD = D // P          # 16 contraction tiles over the model dim
TCH = 512            # token chunk (matmul moving free dim)
NTCH = T // TCH      # 4
NTT = T // P         # 16 token tiles
ROPE_BASE = 10000.0
SCALE = float(HD) ** -0.5
BF16 = ml_dtypes.bfloat16

_CACHE = {}
LAST_RESULT = None


def _build_program():
    import concourse.bacc as bacc
    import concourse.mybir as mybir
    import concourse.tile as tile

    f32 = mybir.dt.float32
    bf16 = mybir.dt.bfloat16
    Exp = mybir.ActivationFunctionType.Exp
    Copy = mybir.ActivationFunctionType.Copy

    nc = bacc.Bacc("TRN2")
    xT = nc.dram_tensor("xT", (D, T), bf16, kind="ExternalInput").ap()
    wqk = nc.dram_tensor("wqk", (D, 2 * NH * HD), bf16, kind="ExternalInput").ap()
    wv = nc.dram_tensor("wv", (D, NH * HD), bf16, kind="ExternalInput").ap()
    wout = nc.dram_tensor("wout", (NH * HD, D), bf16, kind="ExternalInput").ap()
    cosT = nc.dram_tensor("cosT", (P, T), bf16, kind="ExternalInput").ap()
    sinT = nc.dram_tensor("sinT", (P, T), bf16, kind="ExternalInput").ap()
    tri = nc.dram_tensor("tri", (P, P), bf16, kind="ExternalInput").ap()
    iden = nc.dram_tensor("iden", (P, P), bf16, kind="ExternalInput").ap()
    swap = nc.dram_tensor("swap", (P, P), bf16, kind="ExternalInput").ap()
    out = nc.dram_tensor("out", (T, D), f32, kind="ExternalOutput").ap()

    VW = HD + 1          # per-head V width incl. the ones column

    with tile.TileContext(nc) as tc:
        with tc.tile_pool(name="const", bufs=1) as const:
            # Constant preloads ride the gpsimd SWDGE queue so they overlap
            # the x-tile streaming on the sync HWDGE queue; split the big
            # ones so the first k-slices land within a couple of us and the
            # first matmuls can start almost immediately.
            # Preloads spread over three per-engine SWDGE queues (gpsimd /
            # scalar / vector) so they stream in parallel with the x tiles on
            # the sync HWDGE queue.  wqk is split per k-slice so the first
            # matmul can start ~1us in and slices arrive at consumption rate.
            wqk_r = wqk.rearrange("(ko p) c -> p ko c", p=P)
            wqk_sb = const.tile([P, KD, 2 * NH * HD], bf16)
            for k in range(KD):
                # two queues so slices outrun the k-outer consumption rate
                eng = nc.gpsimd if k % 2 == 0 else nc.scalar
                eng.dma_start(wqk_sb[:, k, :], wqk_r[:, k, :])
            cos_sb = const.tile([P, T], bf16)
            nc.scalar.dma_start(cos_sb, cosT)
            sin_sb = const.tile([P, T], bf16)
            nc.scalar.dma_start(sin_sb, sinT)
            tri_sb = const.tile([P, P], bf16)
            nc.scalar.dma_start(tri_sb, tri)
            id_sb = const.tile([P, P], bf16)
            nc.scalar.dma_start(id_sb, iden)
            # wv / wout are loaded lazily (below) so the startup HBM burst is
            # only what the first token-chunk's matmuls actually need.
            wv_r = wv.rearrange("(ko p) c -> p ko c", p=P)
            wv_sb = const.tile([P, KD, NH * HD], bf16)
            wout_sb = const.tile([P, NH, D], bf16)

            qt_sb = const.tile([P, NH, T], bf16)    # Q^T per head (rope'd)
            kt_sb = const.tile([P, NH, T], bf16)    # K^T per head (rope'd)
            vp_sb = const.tile([P, NTT, NH * VW], bf16)  # V' per token tile
            at_sb = const.tile([P, NH, T], bf16)    # attn output transposed

            for h in range(NH):
                nc.vector.memset(vp_sb[:, :, h * VW + HD : h * VW + VW], 1.0)

            # ---------------- Stage A: projections + RoPE ----------------
            with tc.tile_pool(name="xp", bufs=KD + 8) as xpool, \
                 tc.tile_pool(name="qkps", bufs=6, space="PSUM") as qk_ps, \
                 tc.tile_pool(name="vps", bufs=2, space="PSUM") as v_ps, \
                 tc.tile_pool(name="ropet", bufs=2) as rp:
                H2 = HD // 2

                def rope_epilogue(ps, m, tsl):
                    # rotate-half via two partition-shifted DVE multiplies
                    # (sign already folded into sin_sb)
                    cp = rp.tile([P, TCH], f32, tag="cp")
                    nc.vector.tensor_mul(cp, ps, cos_sb[:, tsl])
                    sp = rp.tile([P, TCH], f32, tag="sp")
                    nc.vector.tensor_mul(sp[:H2], ps[H2:], sin_sb[:H2, tsl])
                    nc.vector.tensor_mul(sp[H2:], ps[:H2], sin_sb[H2:, tsl])
                    dst = qt_sb[:, m, tsl] if m < NH else kt_sb[:, m - NH, tsl]
                    nc.vector.tensor_add(dst, sp, cp)

                for tci in range(NTCH):
                    tsl = slice(tci * TCH, (tci + 1) * TCH)
                    xts = []
                    for k in range(KD):
                        xt = xpool.tile([P, TCH], bf16, tag="x")
                        # chunks 0-1: keep the sync HWDGE queue to itself
                        # (gpsimd is still streaming wqk); later chunks split.
                        eng = nc.sync if (tci <= 1 or k % 2 == 0) else nc.gpsimd
                        eng.dma_start(xt, xT[k * P : (k + 1) * P, tsl])
                        xts.append(xt)
                    if tci == 0:
                        # k-outer on the first chunk: consume wqk/x k-slices
                        # in DMA arrival order instead of stalling on the
                        # full 4MB of wqk inside the first m's k-loop.
                        for half in range(2):
                            pss = [
                                qk_ps.tile([P, TCH], f32, tag="qk", name=f"qk{i}")
                                for i in range(NH)
                            ]
                            for k in range(KD):
                                for mi in range(NH):
                                    m = half * NH + mi
                                    nc.tensor.matmul(
                                        pss[mi],
                                        wqk_sb[:, k, m * HD : (m + 1) * HD],
                                        xts[k],
                                        start=(k == 0),
                                        stop=(k == KD - 1),
                                    )
                            for mi in range(NH):
                                rope_epilogue(pss[mi], half * NH + mi, tsl)
                    else:
                        for m in range(2 * NH):
                            ps = qk_ps.tile([P, TCH], f32, tag="qk")
                            for k in range(KD):
                                nc.tensor.matmul(
                                    ps,
                                    wqk_sb[:, k, m * HD : (m + 1) * HD],
                                    xts[k],
                                    start=(k == 0),
                                    stop=(k == KD - 1),
                                )
                            rope_epilogue(ps, m, tsl)
                    if tci == 0:
                        for k4 in range(4):
                            nc.scalar.dma_start(
                                wv_sb[:, k4 * 4 : (k4 + 1) * 4, :],
                                wv_r[:, k4 * 4 : (k4 + 1) * 4, :],
                            )
                    for tt in range(TCH // P):
                        ttg = tci * (TCH // P) + tt
                        psv = v_ps.tile([P, NH * HD], f32, tag="v")
                        for k in range(KD):
                            nc.tensor.matmul(
                                psv,
                                xts[k][:, tt * P : (tt + 1) * P],
                                wv_sb[:, k, :],
                                start=(k == 0),
                                stop=(k == KD - 1),
                            )
                        for h in range(NH):
                            nc.vector.tensor_copy(
                                vp_sb[:, ttg, h * VW : h * VW + HD],
                                psv[:, h * HD : (h + 1) * HD],
                            )

            # -------- Stage B+C: causal attention + output projection --------
            # qc outer, heads inner: once all heads finish a q-chunk, its four
            # token tiles of the output projection are emitted immediately so
            # they overlap the next q-chunk's attention work on the PE.
            # PSUM budget: st 2 + pv/o shared 5 + tp 1 = 8 banks.
            with tc.tile_pool(name="stps", bufs=3, space="PSUM") as st_ps, \
                 tc.tile_pool(name="pvps", bufs=4, space="PSUM") as pv_ps, \
                 tc.tile_pool(name="tpps", bufs=1, space="PSUM") as tp_ps, \
                 tc.tile_pool(name="ptp", bufs=8) as ptp, \
                 tc.tile_pool(name="bt", bufs=8) as bt, \
                 tc.tile_pool(name="ost", bufs=4) as ost:
                for h in range(NH):
                    nc.gpsimd.dma_start(wout_sb[:, h, :], wout[h * P : (h + 1) * P, :])
                for qc in range(NTCH):
                    for h in range(NH):
                        pvp = [
                            pv_ps.tile([P, TCH], f32, tag="pv", name=f"pv{i}")
                            for i in range(TCH // P)
                        ]
                        for ki in range(4 * qc + 4):
                            # diagonal blocks: only q >= ki is causally valid,
                            # so trim the matmul/exp to the valid q suffix
                            j = max(ki - 4 * qc, 0) * P
                            st = st_ps.tile([P, TCH], f32, tag="st")
                            nc.tensor.matmul(
                                st[:, j:],
                                kt_sb[:, h, ki * P : (ki + 1) * P],
                                qt_sb[:, h, qc * TCH + j : (qc + 1) * TCH],
                                start=True,
                                stop=True,
                            )
                            pt = ptp.tile([P, TCH], bf16, tag="pt")
                            nc.scalar.activation(pt[:, j:], st[:, j:], Exp, scale=SCALE)
                            for ql in range(TCH // P):
                                qi = 4 * qc + ql
                                if qi < ki:
                                    continue
                                src = pt[:, ql * P : (ql + 1) * P]
                                if qi == ki:
                                    nc.vector.tensor_mul(src, src, tri_sb)
                                nc.tensor.matmul(
                                    pvp[ql][:, : HD + 1],
                                    src,
                                    vp_sb[:, ki, h * VW : (h + 1) * VW],
                                    start=(ki == 0),
                                    stop=(ki == qi),
                                )
                        for ql in range(TCH // P):
                            qi = 4 * qc + ql
                            invz = bt.tile([P, 1], f32, tag="invz")
                            nc.vector.reciprocal(invz, pvp[ql][:, HD : HD + 1])
                            anorm = bt.tile([P, P], bf16, tag="anorm")
                            nc.vector.tensor_scalar_mul(anorm, pvp[ql][:, :HD], invz)
                            tp = tp_ps.tile([P, P], bf16, tag="tp")
                            nc.tensor.transpose(tp, anorm, id_sb)
                            nc.vector.tensor_copy(at_sb[:, h, qi * P : (qi + 1) * P], tp)
                    for tl in range(TCH // P):
                        tti = 4 * qc + tl
                        for oc in range(D // TCH):
                            po = pv_ps.tile([P, TCH], f32, tag="pv", name="po")
                            for h in range(NH):
                                nc.tensor.matmul(
                                    po,
                                    at_sb[:, h, tti * P : (tti + 1) * P],
                                    wout_sb[:, h, oc * TCH : (oc + 1) * TCH],
                                    start=(h == 0),
                                    stop=(h == NH - 1),
                                )
                            ob = ost.tile([P, TCH], f32, tag="ob")
                            # keep ACT exp-only in this phase: copies on DVE,
                            # DMA triggers on sync/gpsimd (both have slack)
                            nc.vector.tensor_copy(ob, po)
                            eng = nc.sync if oc % 2 == 0 else nc.gpsimd
                            eng.dma_start(
                                out[tti * P : (tti + 1) * P, oc * TCH : (oc + 1) * TCH],
                                ob,
                            )

    nc.compile()
    return nc


def _rope_tables():
    inv_freq = 1.0 / (
        np.float32(ROPE_BASE)
        ** (np.arange(0, HD, 2, dtype=np.float32) / np.float32(HD))
    )
    t = np.arange(T, dtype=np.float32)
    freqs = np.einsum("i,j->ij", t, inv_freq).astype(np.float32)   # (T, HD/2)
    emb = np.concatenate([freqs, freqs], axis=-1)                  # (T, HD)
    cos = np.cos(emb).astype(np.float32)
    sin = np.sin(emb).astype(np.float32)
    cosT = np.ascontiguousarray(cos.T)                             # (HD, T)
    sinT = np.ascontiguousarray(sin.T)
    sinT[: HD // 2] *= -1.0                                        # sign-fold rotate_half
    return cosT, sinT


def _make_in_maps(x, w_qkv, w_out):
    cosT, sinT = _rope_tables()
    kk, qq = np.meshgrid(np.arange(P), np.arange(P), indexing="ij")
    tri = (qq >= kk).astype(BF16)
    iden = np.eye(P, dtype=BF16)
    swap = np.zeros((P, P), dtype=BF16)
    swap[np.arange(P), (np.arange(P) + HD // 2) % P] = 1

    in_maps = []
    for c in range(8):
        b, hg = c // 4, c % 4
        cs = slice(hg * NH * HD, (hg + 1) * NH * HD)
        wq = w_qkv[:, 0 * D : 1 * D][:, cs]
        wk = w_qkv[:, 1 * D : 2 * D][:, cs]
        wv = w_qkv[:, 2 * D : 3 * D][:, cs]
        in_maps.append(
            {
                "xT": np.ascontiguousarray(x[b].T).astype(BF16),
                "wqk": np.ascontiguousarray(
                    np.concatenate([wq, wk], axis=1)
                ).astype(BF16),
                "wv": np.ascontiguousarray(wv).astype(BF16),
                "wout": np.ascontiguousarray(w_out[cs, :]).astype(BF16),
                "cosT": cosT.astype(BF16),
                "sinT": sinT.astype(BF16),
                "tri": tri,
                "iden": iden,
                "swap": swap,
            }
        )
    return in_maps


def _ensure_ntff_hook():
    """bass_utils needs antenv.axon_hooks when tracing under axon; some
    images ship an antenv without it.  Recreate the tiny hook registry and
    register the ctypes-based NTFF capture the boot shim would have."""
    try:
        import antenv.axon_hooks  # noqa: F401
        return
    except ImportError:
        pass
    import sys
    import types

    import antenv

    mod = types.ModuleType("antenv.axon_hooks")
    mod._hook = None

    def set_axon_ntff_profile_hook(h):
        mod._hook = h

    def get_axon_ntff_profile_hook():
        return mod._hook

    mod.set_axon_ntff_profile_hook = set_axon_ntff_profile_hook
    mod.get_axon_ntff_profile_hook = get_axon_ntff_profile_hook
    sys.modules["antenv.axon_hooks"] = mod
    antenv.axon_hooks = mod
    try:
        from trn_agent_boot.trn_boot import _ntff_profile_via_ctypes

        mod._hook = _ntff_profile_via_ctypes("/opt/axon/libaxon_pjrt.so")
    except Exception:
        pass


def kernel(x, w_qkv, w_out):
    global LAST_RESULT
    from concourse import bass_utils

    x = np.asarray(x, dtype=np.float32)
    w_qkv = np.asarray(w_qkv, dtype=np.float32)
    w_out = np.asarray(w_out, dtype=np.float32)

    if "nc" not in _CACHE:
        _CACHE["nc"] = _build_program()
    nc = _CACHE["nc"]

    in_maps = _make_in_maps(x, w_qkv, w_out)
    trace = os.environ.get("KERNEL_TRACE", "0") == "1"
    if trace:
        _ensure_ntff_hook()
    res = bass_utils.run_bass_kernel_spmd(
        nc, in_maps, core_ids=list(range(8)), trace=trace
    )
    LAST_RESULT = res

    out = np.zeros((B, T, D), dtype=np.float32)
    for c in range(8):
        out[c // 4] += res.results[c]["out"]
    return out
